# revision 1
# baseline (speedup 1.0000x reference)
"""Bass/Trainium2 kernel for nn_GCNN_61615600828570 (gated GCNN message passing).

Self-contained: hardcodes shapes/sharding. 8 NeuronCores, sharded as
(batch b, arc-direction) pairs; pair AllReduce between the two GCN blocks.

kernel(**inputs) takes the FULL inputs (numpy, dtypes as in setup_inputs)
and returns the FULL (B, L, E) float32 output.
"""
import numpy as np
import ml_dtypes

import concourse.bass as bass
import concourse.mybir as mybir
import concourse.tile as tile
from concourse import bacc
from concourse.bass_utils import run_bass_kernel_spmd
from concourse.masks import make_identity

F32 = mybir.dt.float32
BF16 = mybir.dt.bfloat16
BF = ml_dtypes.bfloat16

B, L, E, D = 4, 1024, 140, 140
NE, NU, NB = 10, 4, 2
ND = NU + 1
N1 = D + 1            # 141: D outputs + gate column
LT = L // 128         # 8 l-tiles
KT = L // 128         # 8 contraction tiles for arc
E0 = 128              # first x~ k-slab rows
E1 = E + 1 - E0       # 13: remaining e rows + ones row
NCORES = 8
PAIRS = [[0, 1], [2, 3], [4, 5], [6, 7]]
NW = NE * N1          # 1410: all per-type projection columns
P_CHUNKS = [(0, 512), (512, 1024), (1024, NW)]  # psum-bank sized N-chunks
L_GROUPS = [(0, 3), (3, 6), (6, 8)]  # l-tile groups per arc psum bank

_NC = None


def _build(reps=1):
    nc = bacc.Bacc("TRN2", target_bir_lowering=False, debug=False,
                   num_devices=NCORES)

    am_d = nc.dram_tensor("am", [NE, L, L], BF16, kind="ExternalInput")
    x0_d = nc.dram_tensor("x0", [L, E], F32, kind="ExternalInput")
    xt0a_d = nc.dram_tensor("xt0a", [E0, L], BF16, kind="ExternalInput")
    xt0b_d = nc.dram_tensor("xt0b", [16, L], BF16, kind="ExternalInput")
    w0_d = nc.dram_tensor("w0", [NB, E0, NW], BF16, kind="ExternalInput")
    w1_d = nc.dram_tensor("w1", [NB, 16, NW], BF16, kind="ExternalInput")

    outp_d = nc.dram_tensor("outp", [reps, L, E], F32, kind="ExternalOutput")
    x1o_d = nc.dram_tensor("x1o", [reps, L, E], F32, kind="ExternalOutput")

    cc_inA = nc.dram_tensor("cc_inA", [768, E], BF16)
    cc_outA = nc.dram_tensor("cc_outA", [768, E], BF16)
    cc_inB = nc.dram_tensor("cc_inB", [L - 768, E], BF16)
    cc_outB = nc.dram_tensor("cc_outB", [L - 768, E], BF16)

    NRES = 5          # edge types whose adjacency stays SBUF-resident
    KS = 4            # k-slabs per adjacency super-tile (1 MB DMAs)
    COLSLICE = True   # l-group-outer arc loop, column-sliced streaming
    NO_CC = False

    with tile.TileContext(nc) as tc:
        with (
            tc.tile_pool(name="cst", bufs=1) as cst,
            tc.tile_pool(name="amr", bufs=NRES * (KT // KS)) as amr,
            tc.tile_pool(name="ams", bufs=12) as ams,
            tc.tile_pool(name="pp", bufs=8) as ppool,
            tc.tile_pool(name="wp", bufs=2) as wp,
            tc.tile_pool(name="xp", bufs=2) as xp,
            tc.tile_pool(name="gp", bufs=8) as gpool,
            tc.tile_pool(name="psarc", bufs=4, space="PSUM") as psarc,
            tc.tile_pool(name="psmm", bufs=4, space="PSUM") as psmm,
        ):
            ident = cst.tile([128, 128], BF16)
            make_identity(nc, ident[:])

            dma_eng = [nc.sync, nc.scalar]
            am_view = am_d.ap().rearrange("n (j p) c -> n p j c", p=128)

            def load_am_full(n, k4, i):
                t = amr.tile([128, KS * L], BF16, tag="amr",
                             name=f"amr_{n}_{k4}")
                dma_eng[i % 2].dma_start(
                    t[:].rearrange("p (j c) -> p j c", c=L),
                    am_view[n, :, k4 * KS:(k4 + 1) * KS, :])
                return t

            def load_am_full2(pool, n, k4, i):
                t = pool.tile([128, KS * L], BF16, tag="ams",
                              name=f"ams_{n}_{k4}")
                dma_eng[i % 2].dma_start(
                    t[:].rearrange("p (j c) -> p j c", c=L),
                    am_view[n, :, k4 * KS:(k4 + 1) * KS, :])
                return t

            def load_am_cols(n, k4, g0, g1, i):
                w = (g1 - g0) * 128
                t = ams.tile([128, KS * 3 * 128], BF16, tag="ams",
                             name=f"ams_{n}_{k4}")
                dma_eng[i % 2].dma_start(
                    t[:, 0:KS * w].rearrange("p (j c) -> p j c", c=w),
                    am_view[n, :, k4 * KS:(k4 + 1) * KS, g0 * 128:g1 * 128])
                return t

            def emit_p(blk, w0, w1, xt_ap, p_sb, ms, corder):
                seq = ([(c, m) for c in range(3) for m in ms] if corder
                       else [(c, m) for m in ms for c in range(3)])
                for (c, m) in seq:
                    c0, c1 = P_CHUNKS[c]
                    mpool = psmm if m % 2 == 0 else psarc
                    mtag = "pmm" if m % 2 == 0 else "arc"
                    pmm = mpool.tile([128, 512], F32, tag=mtag, name="pmm")
                    nc.tensor.matmul(
                        pmm[:, 0:c1 - c0],
                        xt_ap[:, m * 128:(m + 1) * 128],
                        w0[blk][:, c0:c1], start=True, stop=False)
                    nc.tensor.matmul(
                        pmm[:, 0:c1 - c0],
                        xt_ap[0:E1, L + m * 128:L + (m + 1) * 128],
                        w1[blk][0:E1, c0:c1], start=False, stop=True)
                    if m % 2 == 0:
                        nc.scalar.copy(p_sb[m][:, c0:c1], pmm[:, 0:c1 - c0])
                    else:
                        nc.vector.tensor_copy(p_sb[m][:, c0:c1],
                                              pmm[:, 0:c1 - c0])

            for rep in range(reps):
                # ---- p-phase inputs first so compute starts immediately ----
                xt = xp.tile([128, 2 * L], BF16, tag="xt")
                nc.sync.dma_start(xt[:, 0:L], xt0a_d.ap())
                nc.sync.dma_start(xt[0:16, L:2 * L], xt0b_d.ap())
                w0 = [wp.tile([E0, NW], BF16, tag="w0", name=f"w0_{i}")
                      for i in range(NB)]
                w1 = [wp.tile([16, NW], BF16, tag="w1", name=f"w1_{i}")
                      for i in range(NB)]
                nc.scalar.dma_start(w0[0][:], w0_d.ap()[0])
                nc.scalar.dma_start(w1[0][:], w1_d.ap()[0])

                # adjacency for types 0..NRES-1: resident across both blocks
                am_res = [[load_am_full(n, k4, n * 2 + k4)
                           for k4 in range(KT // KS)] for n in range(NRES)]

                # residual stream + block-1 weights (needed much later)
                xf = xp.tile([128, LT * E], F32, tag="xf")
                nc.sync.dma_start(xf[:].rearrange("p (t d) -> p t d", t=LT),
                                  x0_d.ap().rearrange("(t p) d -> p t d", p=128))
                nc.scalar.dma_start(w0[1][:], w0_d.ap()[1])
                nc.scalar.dma_start(w1[1][:], w1_d.ap()[1])

                p_next = None
                for blk in range(NB):
                    # ---- p~ = x~ @ [W | Wg] for all NE types ----
                    if blk == 0:
                        p_sb = [ppool.tile([128, NW], BF16, tag="p",
                                           name=f"p_0_{i}") for i in range(KT)]
                        emit_p(0, w0, w1, xt, p_sb, range(KT), corder=True)
                    else:
                        p_sb = p_next  # built inside the post-AR chain

                    # ---- arc aggregation ----
                    acc = xp.tile([128, LT * D], F32, tag="acc")
                    nc.gpsimd.memset(acc[:], 0.0)
                    if not COLSLICE:
                        # n-outer, full-slab streaming
                        for n in range(NE):
                            if n < NRES:
                                am_sb = am_res[n]
                            else:
                                am_sb = [load_am_full2(ams, n, k4, n + k4 + blk)
                                         for k4 in range(KT // KS)]
                            for (g0, g1) in L_GROUPS:
                                gl = g1 - g0
                                apool, atag = ((psarc, "arc") if n % 2 == 0
                                               else (psmm, "pmm"))
                                arc = apool.tile([128, 512], F32, tag=atag,
                                                 name="arc")
                                for l in range(g0, g1):
                                    off = (l - g0) * N1
                                    for k in range(KT):
                                        nc.tensor.matmul(
                                            arc[:, off:off + N1],
                                            am_sb[k // KS][:, (k % KS) * L + l * 128:
                                                           (k % KS) * L + (l + 1) * 128],
                                            p_sb[k][:, n * N1:(n + 1) * N1],
                                            start=(k == 0), stop=(k == KT - 1))
                                g_sb = gpool.tile([128, 4], F32, tag="g")
                                nc.scalar.activation(
                                    g_sb[:, 0:gl], arc[:, D:D + (gl - 1) * N1 + 1:N1],
                                    mybir.ActivationFunctionType.Sigmoid)
                                for l in range(g0, g1):
                                    off = (l - g0) * N1
                                    nc.vector.scalar_tensor_tensor(
                                        out=acc[:, l * D:(l + 1) * D],
                                        in0=arc[:, off:off + D],
                                        scalar=g_sb[:, l - g0:l - g0 + 1],
                                        in1=acc[:, l * D:(l + 1) * D],
                                        op0=mybir.AluOpType.mult,
                                        op1=mybir.AluOpType.add)
                        if blk == 0:
                            accb = gpool.tile([128, LT * D], BF16, tag="accb8",
                                              bufs=1, name="accb8")
                            nc.scalar.copy(accb[:], acc[:])
                            nc.gpsimd.dma_start(
                                cc_inA.ap().rearrange("(t p) d -> p t d", p=128),
                                accb[:, 0:6 * D].rearrange("p (t d) -> p t d", d=D))
                            nc.gpsimd.dma_start(
                                cc_inB.ap().rearrange("(t p) d -> p t d", p=128),
                                accb[:, 6 * D:].rearrange("p (t d) -> p t d", d=D))
                            nc.gpsimd.collective_compute(
                                "AllReduce", mybir.AluOpType.add,
                                replica_groups=PAIRS,
                                ins=[cc_inA.ap()], outs=[cc_outA.ap()])
                        else:
                            nc.sync.dma_start(
                                outp_d.ap()[rep].rearrange("(t p) d -> p t d", p=128),
                                acc[:].rearrange("p (t d) -> p t d", t=LT))
                    else:
                     for gi, (g0, g1) in enumerate(L_GROUPS):
                         gl = g1 - g0
                         for n in range(NE):
                             if n < NRES:
                                 def am_ap(k, l, _t=am_res[n]):
                                     return _t[k // KS][:, (k % KS) * L + l * 128:
                                                        (k % KS) * L + (l + 1) * 128]
                             else:
                                 _ts = load_am_cols(n, 0, g0, g1, n + gi + blk)
                                 _ts2 = load_am_cols(n, 1, g0, g1, n + gi + blk + 1)
                                 def am_ap(k, l, _a=_ts, _b=_ts2, _g0=g0, _w=gl * 128):
                                     t = _a if k < KS else _b
                                     return t[:, (k % KS) * _w + (l - _g0) * 128:
                                              (k % KS) * _w + (l - _g0 + 1) * 128]
                             apool, atag = ((psarc, "arc") if n % 2 == 0
                                            else (psmm, "pmm"))
                             arc = apool.tile([128, 512], F32, tag=atag,
                                              name="arc")
                             for l in range(g0, g1):
                                 off = (l - g0) * N1
                                 for k in range(KT):
                                     nc.tensor.matmul(
                                         arc[:, off:off + N1], am_ap(k, l),
                                         p_sb[k][:, n * N1:(n + 1) * N1],
                                         start=(k == 0), stop=(k == KT - 1))
                             g_sb = gpool.tile([128, 4], F32, tag="g")
                             nc.scalar.activation(
                                 g_sb[:, 0:gl], arc[:, D:D + (gl - 1) * N1 + 1:N1],
                                 mybir.ActivationFunctionType.Sigmoid)
                             for l in range(g0, g1):
                                 off = (l - g0) * N1
                                 nc.vector.scalar_tensor_tensor(
                                     out=acc[:, l * D:(l + 1) * D],
                                     in0=arc[:, off:off + D],
                                     scalar=g_sb[:, l - g0:l - g0 + 1],
                                     in1=acc[:, l * D:(l + 1) * D],
                                     op0=mybir.AluOpType.mult,
                                     op1=mybir.AluOpType.add)
                         if blk == 0:
                             # this l-group's partial sum is final: stage it to
                             # the AllReduce bounce buffer while later groups
                             # compute (the collective itself must be a single
                             # full-tensor op -- sliced collectives fall off the
                             # NRT fast path)
                             accb = gpool.tile([128, 3 * D], BF16, tag="accb",
                                               bufs=3, name="accb")
                             nc.scalar.copy(accb[:, 0:(g1 - g0) * D],
                                            acc[:, g0 * D:g1 * D])
                             cdst = (cc_inA.ap()[g0 * 128:g1 * 128, :]
                                     if g1 <= 6 else
                                     cc_inB.ap()[(g0 - 6) * 128:(g1 - 6) * 128, :])
                             nc.gpsimd.dma_start(
                                 cdst.rearrange("(t p) d -> p t d", p=128),
                                 accb[:, 0:(g1 - g0) * D].rearrange(
                                     "p (t d) -> p t d", d=D))
                             if gi == 1:
                                 # rows 0..767 staged: launch their AllReduce
                                 # under l-group-2 compute
                                 nc.gpsimd.collective_compute(
                                     "AllReduce", mybir.AluOpType.add,
                                     replica_groups=PAIRS,
                                     ins=[cc_inA.ap()], outs=[cc_outA.ap()])
                         else:
                             nc.sync.dma_start(
                                 outp_d.ap()[rep, g0 * 128:g1 * 128, :].rearrange(
                                     "(t p) d -> p t d", p=128),
                                 acc[:, g0 * D:g1 * D].rearrange(
                                     "p (t d) -> p t d", d=D))

                    if blk == 0:
                        nc.gpsimd.collective_compute(
                            "AllReduce", mybir.AluOpType.add,
                            replica_groups=PAIRS,
                            ins=[cc_inB.ap()], outs=[cc_outB.ap()])
                        # ---- post-AR chain, pipelined per l-group ----
                        x1 = xp.tile([128, LT * E], F32, tag="xf")
                        x1b = xp.tile([128, LT * E], BF16, tag="x1b")
                        xt_n = xp.tile([128, 2 * L], BF16, tag="xt")
                        nc.gpsimd.memset(xt_n[0:32, L:2 * L], 1.0)
                        red = xp.tile([128, LT * E], BF16, tag="red")
                        for (g0, g1) in L_GROUPS:
                            csrc = (cc_outA.ap()[g0 * 128:g1 * 128, :]
                                    if g1 <= 6 else
                                    cc_outB.ap()[(g0 - 6) * 128:(g1 - 6) * 128, :])
                            nc.gpsimd.dma_start(
                                red[:, g0 * E:g1 * E].rearrange(
                                    "p (t d) -> p t d", d=E),
                                csrc.rearrange("(t p) d -> p t d", p=128))
                            sl = slice(g0 * E, g1 * E)
                            nc.vector.scalar_tensor_tensor(
                                out=x1[:, sl], in0=red[:, sl], scalar=0.0,
                                in1=xf[:, sl], op0=mybir.AluOpType.max,
                                op1=mybir.AluOpType.add)
                            nc.vector.tensor_copy(x1b[:, sl], x1[:, sl])
                            nc.gpsimd.dma_start(
                                x1o_d.ap()[rep, g0 * 128:g1 * 128, :].rearrange(
                                    "(t p) d -> p t d", p=128),
                                x1[:, sl].rearrange("p (t d) -> p t d", d=E))
                            for lt in range(g0, g1):
                                tp = psmm.tile([128, 512], BF16, tag="pmm")
                                nc.tensor.transpose(
                                    tp[:, 0:128],
                                    x1b[:, lt * E:lt * E + 128], ident[:])
                                nc.scalar.copy(
                                    xt_n[:, lt * 128:(lt + 1) * 128], tp[:, 0:128])
                                tp2 = psmm.tile([128, 512], BF16, tag="pmm")
                                nc.tensor.transpose(
                                    tp2[0:E - E0, 0:128],
                                    x1b[:, lt * E + E0:lt * E + E], ident[:])
                                nc.scalar.copy(
                                    xt_n[0:E - E0, L + lt * 128:L + (lt + 1) * 128],
                                    tp2[0:E - E0, 0:128])
                            if g1 == 6:
                                # x1-transposes for l-tiles 0..5 are emitted;
                                # run block-1's p-matmuls for those m-tiles now
                                # so PE works while the second AllReduce drains
                                p_next = [ppool.tile([128, NW], BF16, tag="p",
                                                     name=f"p_1_{i}")
                                          for i in range(KT)]
                                emit_p(1, w0, w1, xt_n, p_next, range(6),
                                       corder=False)
                        emit_p(1, w0, w1, xt_n, p_next, range(6, KT),
                               corder=False)
                        xt = xt_n
                        xf = x1

    nc.compile()
    return nc


def _get_nc():
    global _NC
    if _NC is None:
        _NC = _build()
    return _NC


def _prep_inputs(seq_repr, adj, W_in, b_in, W_out, b_out,
                 Wg_in, bg_in, Wg_out, bg_out):
    """Build the 8 per-core input maps (host-side sharding + layout prep)."""
    et = np.minimum(np.arange(NE), NU)
    seq_repr = np.asarray(seq_repr, np.float32)
    adj = np.asarray(adj)

    # x~0^T slabs, shared by all cores of the same b
    xt_by_b = []
    for b in range(B):
        xt = np.concatenate(
            [seq_repr[b], np.ones((L, 1), np.float32)], axis=1).T  # (141, L)
        xt = xt.astype(BF)
        xt0b = np.zeros((16, L), BF)
        xt0b[0:E1] = xt[E0:E + 1]
        xt_by_b.append((np.ascontiguousarray(xt[0:E0]), xt0b))

    # weight slabs per direction: rows = e (140) + bias row; cols = NE*(D+1)
    def wslabs(Wd, bd, Wgd, bgd):
        w = np.zeros((NB, E + 1, NW), np.float32)
        for blk in range(NB):
            for n in range(NE):
                s = et[n]
                w[blk, 0:E, n * N1:n * N1 + D] = Wd[blk, s]
                w[blk, E, n * N1:n * N1 + D] = bd[blk, s]
                w[blk, 0:E, n * N1 + D] = Wgd[blk, s, :, 0]
                w[blk, E, n * N1 + D] = bgd[blk, s, 0]
        w = w.astype(BF)
        w1 = np.zeros((NB, 16, NW), BF)
        w1[:, 0:E1] = w[:, E0:E + 1]
        return np.ascontiguousarray(w[:, 0:E0]), w1

    w_in0, w_in1 = wslabs(np.asarray(W_in, np.float32), np.asarray(b_in, np.float32),
                          np.asarray(Wg_in, np.float32), np.asarray(bg_in, np.float32))
    w_out0, w_out1 = wslabs(np.asarray(W_out, np.float32), np.asarray(b_out, np.float32),
                            np.asarray(Wg_out, np.float32), np.asarray(bg_out, np.float32))

    in_maps = []
    for c in range(NCORES):
        b, dirn = c // 2, c % 2
        a = adj[b].astype(BF)  # (NE, L, L)
        if dirn == 0:
            # in-arcs: lhsT tile [m, l] must hold A[l, m] -> transpose
            am = np.ascontiguousarray(a.transpose(0, 2, 1))
            w0, w1 = w_in0, w_in1
        else:
            am = np.ascontiguousarray(a)
            w0, w1 = w_out0, w_out1
        xt0a, xt0b = xt_by_b[b]
        in_maps.append({
            "am": am, "x0": np.ascontiguousarray(seq_repr[b]),
            "xt0a": xt0a, "xt0b": xt0b, "w0": w0, "w1": w1,
        })
    return in_maps


def _combine(results):
    """Host epilogue: x2 = relu(p_in + p_out) + x1 per batch."""
    out = np.empty((B, L, E), np.float32)
    for b in range(B):
        pin = results[2 * b]["outp"][0]
        pout = results[2 * b + 1]["outp"][0]
        x1 = results[2 * b]["x1o"][0]
        out[b] = np.maximum(pin + pout, 0.0) + x1
    return out


def run_on_hw(in_maps, trace=False, **kw):
    nc = _get_nc()
    res = run_bass_kernel_spmd(nc, in_maps, core_ids=list(range(NCORES)),
                               trace=trace, **kw)
    return res


def kernel(**inputs):
    in_maps = _prep_inputs(**inputs)
    res = run_on_hw(in_maps)
    return _combine(res.results)



# revision 41
# speedup vs baseline: 2.8182x; 2.8182x over previous
"""Bass/Trainium2 kernel for nn_GCNN_61615600828570 (gated GCNN message passing).

Self-contained: hardcodes shapes/sharding. 8 NeuronCores, sharded as
(batch b, arc-direction) pairs; single pair AllGather between the two GCN
blocks (each core then sums both partials locally).

Arc aggregation runs in fp8 (float8e4) DoubleRow perf mode at 2x PE rate:
the 0/1 adjacency is exact in fp8, and the per-type projections p are split
hi+lo (p ~= fp8(p) + fp8(p - fp8(p))), giving bf16-level accuracy at fp8
speed. Block-1 projections are pre-scaled by 1/8 to stay inside e4m3 range
(max 240); the scale is undone in the gate sigmoid (scale=8) and on the
host for the block-1 output partials.

kernel(**inputs) takes the FULL inputs (numpy, dtypes as in setup_inputs)
and returns the FULL (B, L, E) float32 output.
"""
import numpy as np
import ml_dtypes

import concourse.bass as bass
import concourse.mybir as mybir
import concourse.tile as tile
from concourse import bacc
from concourse.bass_utils import run_bass_kernel_spmd

F32 = mybir.dt.float32
BF16 = mybir.dt.bfloat16
FP8 = mybir.dt.float8e4
BF = ml_dtypes.bfloat16
E4 = ml_dtypes.float8_e4m3

B, L, E, D = 4, 1024, 140, 140
NE, NU, NB = 10, 4, 2
ND = NU + 1
N1 = D + 1            # 141: D outputs + gate column
LT = L // 128         # 8 l-tiles
KT = L // 128         # 8 contraction tiles for arc
E0 = 128              # first x~ k-slab rows
E1 = E + 1 - E0       # 13: remaining e rows + ones row
NCORES = 8
PAIRS = [[0, 1], [2, 3], [4, 5], [6, 7]]
NW = ND * N1          # 705: distinct projection columns (types >= NU share
                      # weight set NU, so only ND=5 column groups are needed)
P_CHUNKS = [(0, 512), (512, NW)]  # psum-bank sized N-chunks
L_GROUPS = [(0, 3), (3, 6), (6, 8)]  # l-tile groups per arc psum bank
KS = 4                # k-slabs per adjacency super-tile (1 MB DMAs)
S1 = 8.0              # block-1 fp8 pre-scale (keeps |p|/S1 << e4m3 max 240)
DR = mybir.MatmulPerfMode.DoubleRow

_NC = None


# build-time scheduling knobs (tuned against TimelineSim)
CFG = {
    "early_q": "scalar",   # "split": xt q0 sync + rest scalar; "scalar": all sync/scalar
    "xf_q": "gpsimd",      # "sync_late" | "gpsimd"
    "interleave": "1x1",   # "2x2": c0 a0 a1 c1 a2 a3 c2 | "1x1": c0 a0 c1 a1 a2 c2
    "par_flip": False,     # flip gating parity for last two types
    "segs": "groups",      # "fine": (0,1)(1,3)(3,6)(6,8) | "groups"
    "t_alt": True,         # alternate transpose queue sync/scalar
    "outp_tile": False,    # per-l-tile outp DMA for last group
}
# edge types whose gating goes through the SBUF-staged gpsimd path
PB_TYPES = {1, 3, 5, 7, 8}


def _build(reps=1):
    nc = bacc.Bacc("TRN2", target_bir_lowering=False, debug=False,
                   num_devices=NCORES)

    am_d = nc.dram_tensor("am", [NE, L, L], FP8, kind="ExternalInput")
    x0_d = nc.dram_tensor("x0", [L, E], F32, kind="ExternalInput")
    xt0a_d = nc.dram_tensor("xt0a", [E0, L], BF16, kind="ExternalInput")
    xt0b_d = nc.dram_tensor("xt0b", [16, L], BF16, kind="ExternalInput")
    w0_d = nc.dram_tensor("w0", [NB, E0, NW], BF16, kind="ExternalInput")
    w1_d = nc.dram_tensor("w1", [NB, 16, NW], BF16, kind="ExternalInput")

    outp_d = nc.dram_tensor("outp", [reps, L, E], F32, kind="ExternalOutput")
    x1o_d = nc.dram_tensor("x1o", [reps, L, E], BF16, kind="ExternalOutput")

    cc_in = nc.dram_tensor("cc_in", [L, E], BF16)
    cc_out = nc.dram_tensor("cc_out", [2 * L, E], BF16)

    with tile.TileContext(nc) as tc:
        with (
            tc.tile_pool(name="amr", bufs=NE * (KT // KS)) as amr,
            tc.tile_pool(name="pp", bufs=KT // 2) as ppool,
            tc.tile_pool(name="wp", bufs=2) as wp,
            tc.tile_pool(name="xp", bufs=2) as xp,
            tc.tile_pool(name="gp", bufs=8) as gpool,
            tc.tile_pool(name="psarc", bufs=4, space="PSUM") as psarc,
            tc.tile_pool(name="psmm", bufs=4, space="PSUM") as psmm,
        ):
            am_view = am_d.ap().rearrange("n (j p) c -> n p j c", p=128)

            def load_am(n, half):
                t = amr.tile([128, KS * L], FP8, tag="amr",
                             name=f"amr_{n}_{half}")
                nc.sync.dma_start(
                    t[:].rearrange("p (j c) -> p j c", c=L),
                    am_view[n, :, half * KS:(half + 1) * KS, :])
                return t

            def am_pair(am_res, n, a, l):
                # lhsT AP [128, 2, 128]: k-slab pair (2a, 2a+1), l-tile cols
                t = am_res[n][a // 2]
                return t[:].rearrange("p (j c) -> p j c", c=L)[
                    :, 2 * (a % 2):2 * (a % 2) + 2, l * 128:(l + 1) * 128]

            def p_pair(psrc, a, n):
                # rhs AP [128, 2, N1]: k-slab pair (2a, 2a+1), cols of the
                # weight set shared by type n
                s = min(n, NU)
                return psrc[a][:].rearrange("p (j c) -> p j c", c=NW)[
                    :, :, s * N1:(s + 1) * N1]

            def emit_p(blk, w0, w1, xt_ap, ph, pl, ms, corder, cs=None):
                if cs is None:
                    cs = range(len(P_CHUNKS))
                inv_s = 1.0 if blk == 0 else 1.0 / S1
                seq = ([(c, m) for c in cs for m in ms] if corder
                       else [(c, m) for m in ms for c in cs])
                for (c, m) in seq:
                    c0, c1 = P_CHUNKS[c]
                    mpool = psmm if m % 2 == 0 else psarc
                    mtag = "pmm" if m % 2 == 0 else "arc"
                    pmm = mpool.tile([128, 512], F32, tag=mtag, name="pmm")
                    nc.tensor.matmul(
                        pmm[:, 0:c1 - c0],
                        xt_ap[:, m * 128:(m + 1) * 128],
                        w0[blk][:, c0:c1], start=True, stop=False)
                    nc.tensor.matmul(
                        pmm[:, 0:c1 - c0],
                        xt_ap[0:E1, L + m * 128:L + (m + 1) * 128],
                        w1[blk][0:E1, c0:c1], start=False, stop=True)
                    po = (m % 2) * NW
                    hi = ph[m // 2][:, po + c0:po + c1]
                    if blk == 0:
                        nc.scalar.copy(hi, pmm[:, 0:c1 - c0])
                    else:
                        nc.scalar.activation(
                            hi, pmm[:, 0:c1 - c0],
                            mybir.ActivationFunctionType.Copy, scale=inv_s)
                    # lo-residual quantize (DVE: gpsimd cannot access PSUM)
                    nc.vector.scalar_tensor_tensor(
                        out=pl[m // 2][:, po + c0:po + c1],
                        in0=pmm[:, 0:c1 - c0], scalar=inv_s, in1=hi,
                        op0=mybir.AluOpType.mult,
                        op1=mybir.AluOpType.subtract)

            def emit_arc(blk, n, am_res, ph, pl, acc, rep):
                """All l-groups of one edge type: DoubleRow hi+lo chains,
                gate sigmoid, gated accumulate. For the last type the gated
                accumulate writes straight to the staging tile / out DMA."""
                sgate = 1.0 if blk == 0 else S1
                last = n == NE - 1
                for gi, (g0, g1) in enumerate(L_GROUPS):
                    gl = g1 - g0
                    apool, atag = ((psarc, "arc") if (n + gi) % 2 == 0
                                   else (psmm, "pmm"))
                    arc = apool.tile([128, 512], F32, tag=atag, name="arc")
                    for l in range(g0, g1):
                        off = (l - g0) * N1
                        for h, psrc in ((0, ph), (1, pl)):
                            for a in range(KT // 2):
                                nc.tensor.matmul(
                                    arc[:, off:off + N1],
                                    am_pair(am_res, n, a, l),
                                    p_pair(psrc, a, n),
                                    start=(h == 0 and a == 0),
                                    stop=(h == 1 and a == KT // 2 - 1),
                                    perf_mode=DR)
                    g_sb = gpool.tile([128, 4], F32, tag="g")
                    nc.scalar.activation(
                        g_sb[:, 0:gl], arc[:, D:D + (gl - 1) * N1 + 1:N1],
                        mybir.ActivationFunctionType.Sigmoid, scale=sgate)
                    # gpsimd cannot touch PSUM (and has no TensorScalarPtr):
                    # path A gates straight from psum on DVE; path B has Act
                    # apply the gate during the psum->SBUF copy and Pool do
                    # the accumulate from SBUF (3-engine spread)
                    path_b = (3 * n + gi) % 9 < 4 and not last
                    if path_b:
                        for l in range(g0, g1):
                            off = (l - g0) * N1
                            t = gpool.tile([128, D], F32, tag="gt", bufs=6,
                                           name="gt")
                            nc.scalar.activation(
                                t[:], arc[:, off:off + D],
                                mybir.ActivationFunctionType.Copy,
                                scale=g_sb[:, l - g0:l - g0 + 1])
                            nc.gpsimd.tensor_tensor(
                                out=acc[:, l * D:(l + 1) * D],
                                in0=acc[:, l * D:(l + 1) * D],
                                in1=t[:], op=mybir.AluOpType.add)
                        continue
                    gsrc, geng = arc, nc.vector
                    if last and blk == 0:
                        # final gated accumulate writes bf16 staging directly
                        accb = gpool.tile([128, 3 * D], BF16, tag="accb",
                                          bufs=3, name="accb")
                        for l in range(g0, g1):
                            off = (l - g0) * N1
                            geng.scalar_tensor_tensor(
                                out=accb[:, (l - g0) * D:(l - g0 + 1) * D],
                                in0=gsrc[:, off:off + D],
                                scalar=g_sb[:, l - g0:l - g0 + 1],
                                in1=acc[:, l * D:(l + 1) * D],
                                op0=mybir.AluOpType.mult,
                                op1=mybir.AluOpType.add)
                        nc.sync.dma_start(
                            cc_in.ap()[g0 * 128:g1 * 128, :]
                            .rearrange("(t p) d -> p t d", p=128),
                            accb[:, 0:gl * D].rearrange(
                                "p (t d) -> p t d", d=D))
                    else:
                        for l in range(g0, g1):
                            off = (l - g0) * N1
                            geng.scalar_tensor_tensor(
                                out=acc[:, l * D:(l + 1) * D],
                                in0=gsrc[:, off:off + D],
                                scalar=g_sb[:, l - g0:l - g0 + 1],
                                in1=acc[:, l * D:(l + 1) * D],
                                op0=mybir.AluOpType.mult,
                                op1=mybir.AluOpType.add)
                        if last:
                            # last group: per-l-tile DMAs to shorten the tail
                            nl = (1 if CFG["outp_tile"]
                                  and gi == len(L_GROUPS) - 1 else gl)
                            for (q0, q1) in ([(g0, g1)] if nl == gl else
                                             [(l, l + 1) for l in range(g0, g1)]):
                                nc.sync.dma_start(
                                    outp_d.ap()[rep, q0 * 128:q1 * 128, :]
                                    .rearrange("(t p) d -> p t d", p=128),
                                    acc[:, q0 * D:q1 * D].rearrange(
                                        "p (t d) -> p t d", d=D))

            for rep in range(reps):
                # ---- first p-matmul deps lead each queue (xt chunk 0 on
                # sync, w0 chunk 0 on scalar), then the adjacency stream owns
                # sync while the remaining small inputs trickle on scalar ----
                xt = xp.tile([128, 2 * L], BF16, tag="xt")
                w0 = [wp.tile([E0, NW], BF16, tag="w0", name=f"w0_{i}")
                      for i in range(NB)]
                w1 = [wp.tile([16, NW], BF16, tag="w1", name=f"w1_{i}")
                      for i in range(NB)]
                if CFG["early_q"] == "split":
                    nc.sync.dma_start(xt[:, 0:256], xt0a_d.ap()[:, 0:256])
                    nc.scalar.dma_start(w0[0][:, 0:512], w0_d.ap()[0][:, 0:512])
                    am_res = [[load_am(n, half) for half in range(KT // KS)]
                              for n in range(NE)]
                    for q in range(1, 4):
                        nc.scalar.dma_start(xt[:, q * 256:(q + 1) * 256],
                                            xt0a_d.ap()[:, q * 256:(q + 1) * 256])
                    nc.scalar.dma_start(xt[0:16, L:2 * L], xt0b_d.ap())
                    for (c0, c1) in P_CHUNKS[1:]:
                        nc.scalar.dma_start(w0[0][:, c0:c1],
                                            w0_d.ap()[0][:, c0:c1])
                    nc.scalar.dma_start(w1[0][:], w1_d.ap()[0])
                else:
                    for q in range(4):
                        nc.sync.dma_start(xt[:, q * 256:(q + 1) * 256],
                                          xt0a_d.ap()[:, q * 256:(q + 1) * 256])
                    nc.sync.dma_start(xt[0:16, L:2 * L], xt0b_d.ap())
                    for (c0, c1) in P_CHUNKS:
                        nc.scalar.dma_start(w0[0][:, c0:c1],
                                            w0_d.ap()[0][:, c0:c1])
                    nc.scalar.dma_start(w1[0][:], w1_d.ap()[0])
                    am_res = [[load_am(n, half) for half in range(KT // KS)]
                              for n in range(NE)]

                # residual stream + block-1 weights (needed much later)
                xf = xp.tile([128, LT * E], F32, tag="xf")
                xf_eng = nc.sync if CFG["xf_q"] == "sync_late" else nc.gpsimd
                xf_eng.dma_start(xf[:].rearrange("p (t d) -> p t d", t=LT),
                                 x0_d.ap().rearrange("(t p) d -> p t d", p=128))
                xf_eng.dma_start(w0[1][:], w0_d.ap()[1])
                xf_eng.dma_start(w1[1][:], w1_d.ap()[1])

                # ---- block 0: p-chunks interleaved with arc types so the PE
                # stays busy while the adjacency streams in ----
                acc = xp.tile([128, LT * D], F32, tag="acc")
                nc.gpsimd.memset(acc[:], 0.0)
                ph = [ppool.tile([128, 2 * NW], FP8, tag="ph",
                                 name=f"ph_0_{a}") for a in range(4)]
                pl = [ppool.tile([128, 2 * NW], FP8, tag="pl",
                                 name=f"pl_0_{a}") for a in range(4)]
                # p-chunk c0 covers weight sets 0-2; c1 completes sets 3-4
                emit_p(0, w0, w1, xt, ph, pl, range(KT), corder=True, cs=[0])
                emit_arc(0, 0, am_res, ph, pl, acc, rep)
                emit_arc(0, 1, am_res, ph, pl, acc, rep)
                emit_p(0, w0, w1, xt, ph, pl, range(KT), corder=True, cs=[1])
                emit_arc(0, 2, am_res, ph, pl, acc, rep)
                for n in range(3, NE):
                    emit_arc(0, n, am_res, ph, pl, acc, rep)

                # single pair AllGather of the full bf16 partial; each core
                # then sums both halves locally (one fixed overhead, no
                # AllReduce 1.875x factor)
                nc.gpsimd.collective_compute(
                    "AllGather", mybir.AluOpType.bypass,
                    replica_groups=PAIRS,
                    ins=[cc_in.ap()], outs=[cc_out.ap()])

                # ---- post-AG chain ----
                # x1b layout: 256 cols per l-tile (140 data + ones col + junk)
                # so one DMA xbar transpose per tile lands both the main
                # 128-row slab and the 13-row tail of x~1^T
                x1 = xp.tile([128, LT * E], F32, tag="xf")
                x1b = xp.tile([128, LT * 256], BF16, tag="x1b")
                x1b3 = x1b[:].rearrange("p (t c) -> p t c", c=256)
                nc.gpsimd.memset(x1b3[:, :, 140:256], 1.0)
                xt_n = xp.tile([128, 2 * L], BF16, tag="xt")
                red = xp.tile([128, LT * E], BF16, tag="red")
                redb = xp.tile([128, LT * E], BF16, tag="redb")
                ph_next = [ppool.tile([128, 2 * NW], FP8, tag="ph",
                                      name=f"ph_1_{a}") for a in range(4)]
                pl_next = [ppool.tile([128, 2 * NW], FP8, tag="pl",
                                      name=f"pl_1_{a}") for a in range(4)]
                xf3 = xf[:].rearrange("p (t c) -> p t c", c=E)
                # red/redb first (they only depend on the AG) so no other
                # queue work head-of-line blocks them; tile 0 alone first so
                # the first transpose + p-matmuls start ASAP
                SEGS = ([(0, 1), (1, 3), (3, 6), (6, 8)]
                        if CFG["segs"] == "fine" else list(L_GROUPS))
                for (g0, g1) in SEGS:
                    sl = slice(g0 * E, g1 * E)
                    nc.sync.dma_start(
                        red[:, sl].rearrange("p (t d) -> p t d", d=E),
                        cc_out.ap()[g0 * 128:g1 * 128, :].rearrange(
                            "(t p) d -> p t d", p=128))
                    nc.gpsimd.dma_start(
                        redb[:, sl].rearrange("p (t d) -> p t d", d=E),
                        cc_out.ap()[L + g0 * 128:L + g1 * 128, :]
                        .rearrange("(t p) d -> p t d", p=128))
                for (g0, g1) in SEGS:
                    sl = slice(g0 * E, g1 * E)
                    nc.vector.tensor_tensor(
                        out=x1[:, sl], in0=red[:, sl], in1=redb[:, sl],
                        op=mybir.AluOpType.add)
                    nc.vector.scalar_tensor_tensor(
                        out=x1b3[:, g0:g1, 0:E], in0=x1[:, sl], scalar=0.0,
                        in1=xf3[:, g0:g1, :], op0=mybir.AluOpType.max,
                        op1=mybir.AluOpType.add)
                    nc.gpsimd.dma_start(
                        x1o_d.ap()[rep, g0 * 128:g1 * 128, :].rearrange(
                            "(t p) d -> p t d", p=128),
                        x1b3[:, g0:g1, 0:E])
                    for lt in range(g0, g1):
                        # one xbar transpose: cols 0:128 -> x~1^T main slab,
                        # cols 128:256 -> tail slab (rows 13.. are junk that
                        # the 13-row contraction never reads); alternate the
                        # issue queue so issue overhead doesn't starve the PE
                        teng = (nc.sync if lt % 2 == 0 or not CFG["t_alt"]
                                else nc.scalar)
                        teng.dma_start_transpose(
                            xt_n[:].rearrange("p (j c) -> p j c", c=L)[
                                :, :, lt * 128:(lt + 1) * 128],
                            x1b[:, lt * 256:(lt + 1) * 256])
                        emit_p(1, w0, w1, xt_n, ph_next, pl_next, [lt],
                               corder=False)

                # ---- block 1 arcs ----
                acc1 = xp.tile([128, LT * D], F32, tag="acc")
                nc.gpsimd.memset(acc1[:], 0.0)
                for n in range(NE):
                    emit_arc(1, n, am_res, ph_next, pl_next, acc1, rep)

    nc.compile()
    return nc


def _get_nc():
    global _NC
    if _NC is None:
        _NC = _build()
    return _NC


def _prep_inputs(seq_repr, adj, W_in, b_in, W_out, b_out,
                 Wg_in, bg_in, Wg_out, bg_out):
    """Build the 8 per-core input maps (host-side sharding + layout prep)."""
    et = np.minimum(np.arange(NE), NU)
    seq_repr = np.asarray(seq_repr, np.float32)
    adj = np.asarray(adj)

    # x~0^T slabs, shared by all cores of the same b
    xt_by_b = []
    for b in range(B):
        xt = np.concatenate(
            [seq_repr[b], np.ones((L, 1), np.float32)], axis=1).T  # (141, L)
        xt = xt.astype(BF)
        xt0b = np.zeros((16, L), BF)
        xt0b[0:E1] = xt[E0:E + 1]
        xt_by_b.append((np.ascontiguousarray(xt[0:E0]), xt0b))

    # weight slabs per direction: rows = e (140) + bias row; cols = ND*(D+1)
    # (only the ND distinct weight sets -- shared types reuse set NU)
    def wslabs(Wd, bd, Wgd, bgd):
        w = np.zeros((NB, E + 1, NW), np.float32)
        for blk in range(NB):
            for s in range(ND):
                w[blk, 0:E, s * N1:s * N1 + D] = Wd[blk, s]
                w[blk, E, s * N1:s * N1 + D] = bd[blk, s]
                w[blk, 0:E, s * N1 + D] = Wgd[blk, s, :, 0]
                w[blk, E, s * N1 + D] = bgd[blk, s, 0]
        w = w.astype(BF)
        w1 = np.zeros((NB, 16, NW), BF)
        w1[:, 0:E1] = w[:, E0:E + 1]
        return np.ascontiguousarray(w[:, 0:E0]), w1

    w_in0, w_in1 = wslabs(np.asarray(W_in, np.float32), np.asarray(b_in, np.float32),
                          np.asarray(Wg_in, np.float32), np.asarray(bg_in, np.float32))
    w_out0, w_out1 = wslabs(np.asarray(W_out, np.float32), np.asarray(b_out, np.float32),
                            np.asarray(Wg_out, np.float32), np.asarray(bg_out, np.float32))

    in_maps = []
    for c in range(NCORES):
        b, dirn = c // 2, c % 2
        a = adj[b]  # (NE, L, L) int32
        if dirn == 0:
            # in-arcs: lhsT tile [m, l] must hold A[l, m] -> transpose
            am = np.ascontiguousarray(a.transpose(0, 2, 1)).astype(E4)
            w0, w1 = w_in0, w_in1
        else:
            am = np.ascontiguousarray(a).astype(E4)
            w0, w1 = w_out0, w_out1
        xt0a, xt0b = xt_by_b[b]
        in_maps.append({
            "am": am, "x0": np.ascontiguousarray(seq_repr[b]),
            "xt0a": xt0a, "xt0b": xt0b, "w0": w0, "w1": w1,
        })
    return in_maps


def _combine(results):
    """Host epilogue: x2 = relu(S1*(p_in + p_out)) + x1 per batch."""
    out = np.empty((B, L, E), np.float32)
    for b in range(B):
        pin = results[2 * b]["outp"][0]
        pout = results[2 * b + 1]["outp"][0]
        x1 = results[2 * b]["x1o"][0].astype(np.float32)
        out[b] = np.maximum((pin + pout) * S1, 0.0) + x1
    return out


def run_on_hw(in_maps, trace=False, **kw):
    nc = _get_nc()
    res = run_bass_kernel_spmd(nc, in_maps, core_ids=list(range(NCORES)),
                               trace=trace, **kw)
    return res


def kernel(**inputs):
    in_maps = _prep_inputs(**inputs)
    res = run_on_hw(in_maps)
    return _combine(res.results)


# revision 53
# speedup vs baseline: 2.8879x; 1.0247x over previous
"""Bass/Trainium2 kernel for nn_GCNN_61615600828570 (gated GCNN message passing).

Self-contained: hardcodes shapes/sharding. 8 NeuronCores, sharded as
(batch b, arc-direction) pairs; single pair AllGather between the two GCN
blocks (each core then sums both partials locally).

Arc aggregation runs in fp8 (float8e4) DoubleRow perf mode at 2x PE rate:
the 0/1 adjacency is exact in fp8, and the per-type projections p are split
hi+lo (p ~= fp8(p) + fp8(p - fp8(p))), giving bf16-level accuracy at fp8
speed. Block-1 projections are pre-scaled by 1/8 to stay inside e4m3 range
(max 240); the scale is undone in the gate sigmoid (scale=8) and on the
host for the block-1 output partials.

kernel(**inputs) takes the FULL inputs (numpy, dtypes as in setup_inputs)
and returns the FULL (B, L, E) float32 output.
"""
import numpy as np
import ml_dtypes

import concourse.bass as bass
import concourse.mybir as mybir
import concourse.tile as tile
from concourse import bacc
from concourse.bass_utils import run_bass_kernel_spmd

F32 = mybir.dt.float32
BF16 = mybir.dt.bfloat16
FP8 = mybir.dt.float8e4
BF = ml_dtypes.bfloat16
E4 = ml_dtypes.float8_e4m3

B, L, E, D = 4, 1024, 140, 140
NE, NU, NB = 10, 4, 2
ND = NU + 1
N1 = D + 1            # 141: D outputs + gate column
LT = L // 128         # 8 l-tiles
KT = L // 128         # 8 contraction tiles for arc
E0 = 128              # first x~ k-slab rows
E1 = E + 1 - E0       # 13: remaining e rows + ones row
NCORES = 8
PAIRS = [[0, 1], [2, 3], [4, 5], [6, 7]]
NW = ND * N1          # 705: distinct projection columns (types >= NU share
                      # weight set NU, so only ND=5 column groups are needed)
P_CHUNKS = [(0, 512), (512, NW)]  # psum-bank sized N-chunks
L_GROUPS = [(0, 3), (3, 6), (6, 8)]  # l-tile groups per arc psum bank
KS = 4                # k-slabs per adjacency super-tile (1 MB DMAs)
S1 = 8.0              # block-1 fp8 pre-scale (keeps |p|/S1 << e4m3 max 240)
DR = mybir.MatmulPerfMode.DoubleRow

_NC = None


# build-time scheduling knobs (tuned against TimelineSim)
CFG = {
    "early_q": "scalar",   # "split": xt q0 sync + rest scalar; "scalar": all sync/scalar
    "xf_q": "gpsimd",      # "sync_late" | "gpsimd"
    "interleave": "1x1",   # "2x2": c0 a0 a1 c1 a2 a3 c2 | "1x1": c0 a0 c1 a1 a2 c2
    "par_flip": False,     # flip gating parity for last two types
    "segs": "groups",      # "fine": (0,1)(1,3)(3,6)(6,8) | "groups"
    "t_alt": True,         # alternate transpose queue sync/scalar
    "outp_tile": False,    # per-l-tile outp DMA for last group
    "tail": "g1b",         # (unused with split accumulators)
    "pboth": True,         # emit both block-0 p-chunks before the arcs
    "bsel": "9_4",         # B-path unit selector: 9_4 | 5_3 | 3_1
}
# edge types whose gating goes through the SBUF-staged gpsimd path
PB_TYPES = {1, 3, 5, 7, 8}


def _build(reps=1):
    nc = bacc.Bacc("TRN2", target_bir_lowering=False, debug=False,
                   num_devices=NCORES)

    am_d = nc.dram_tensor("am", [NE, L, L], FP8, kind="ExternalInput")
    x0_d = nc.dram_tensor("x0", [L, E], F32, kind="ExternalInput")
    xt0a_d = nc.dram_tensor("xt0a", [E0, L], BF16, kind="ExternalInput")
    xt0b_d = nc.dram_tensor("xt0b", [16, L], BF16, kind="ExternalInput")
    w0_d = nc.dram_tensor("w0", [NB, E0, NW], BF16, kind="ExternalInput")
    w1_d = nc.dram_tensor("w1", [NB, 16, NW], BF16, kind="ExternalInput")

    outp_d = nc.dram_tensor("outp", [reps, L, E], F32, kind="ExternalOutput")
    x1o_d = nc.dram_tensor("x1o", [reps, L, E], BF16, kind="ExternalOutput")

    cc_in = nc.dram_tensor("cc_in", [L, E], BF16)
    cc_out = nc.dram_tensor("cc_out", [2 * L, E], BF16)

    with tile.TileContext(nc) as tc:
        with (
            tc.tile_pool(name="amr", bufs=NE * (KT // KS)) as amr,
            tc.tile_pool(name="pp", bufs=KT // 2) as ppool,
            tc.tile_pool(name="wp", bufs=2) as wp,
            tc.tile_pool(name="xp", bufs=2) as xp,
            tc.tile_pool(name="gp", bufs=8) as gpool,
            tc.tile_pool(name="psarc", bufs=4, space="PSUM") as psarc,
            tc.tile_pool(name="psmm", bufs=4, space="PSUM") as psmm,
        ):
            am_view = am_d.ap().rearrange("n (j p) c -> n p j c", p=128)

            def load_am(n, half):
                t = amr.tile([128, KS * L], FP8, tag="amr",
                             name=f"amr_{n}_{half}")
                nc.sync.dma_start(
                    t[:].rearrange("p (j c) -> p j c", c=L),
                    am_view[n, :, half * KS:(half + 1) * KS, :])
                return t

            def am_pair(am_res, n, a, l):
                # lhsT AP [128, 2, 128]: k-slab pair (2a, 2a+1), l-tile cols
                t = am_res[n][a // 2]
                return t[:].rearrange("p (j c) -> p j c", c=L)[
                    :, 2 * (a % 2):2 * (a % 2) + 2, l * 128:(l + 1) * 128]

            def p_pair(psrc, a, n):
                # rhs AP [128, 2, N1]: k-slab pair (2a, 2a+1), cols of the
                # weight set shared by type n
                s = min(n, NU)
                return psrc[a][:].rearrange("p (j c) -> p j c", c=NW)[
                    :, :, s * N1:(s + 1) * N1]

            def emit_p(blk, w0, w1, xt_ap, ph, pl, ms, corder, cs=None):
                if cs is None:
                    cs = range(len(P_CHUNKS))
                inv_s = 1.0 if blk == 0 else 1.0 / S1
                seq = ([(c, m) for c in cs for m in ms] if corder
                       else [(c, m) for m in ms for c in cs])
                for (c, m) in seq:
                    c0, c1 = P_CHUNKS[c]
                    mpool = psmm if m % 2 == 0 else psarc
                    mtag = "pmm" if m % 2 == 0 else "arc"
                    pmm = mpool.tile([128, 512], F32, tag=mtag, name="pmm")
                    nc.tensor.matmul(
                        pmm[:, 0:c1 - c0],
                        xt_ap[:, m * 128:(m + 1) * 128],
                        w0[blk][:, c0:c1], start=True, stop=False)
                    nc.tensor.matmul(
                        pmm[:, 0:c1 - c0],
                        xt_ap[0:E1, L + m * 128:L + (m + 1) * 128],
                        w1[blk][0:E1, c0:c1], start=False, stop=True)
                    po = (m % 2) * NW
                    hi = ph[m // 2][:, po + c0:po + c1]
                    if blk == 0:
                        nc.scalar.copy(hi, pmm[:, 0:c1 - c0])
                    else:
                        nc.scalar.activation(
                            hi, pmm[:, 0:c1 - c0],
                            mybir.ActivationFunctionType.Copy, scale=inv_s)
                    # lo-residual quantize (DVE: gpsimd cannot access PSUM)
                    nc.vector.scalar_tensor_tensor(
                        out=pl[m // 2][:, po + c0:po + c1],
                        in0=pmm[:, 0:c1 - c0], scalar=inv_s, in1=hi,
                        op0=mybir.AluOpType.mult,
                        op1=mybir.AluOpType.subtract)

            def emit_arc(blk, n, am_res, ph, pl, accA, accB, rep):
                """All l-groups of one edge type: DoubleRow hi+lo chains,
                gate sigmoid, gated accumulate. Two independent accumulator
                chains (A: DVE straight from psum; B: Act gate-scaled copy +
                Pool add from SBUF) merged per group after the last type."""
                sgate = 1.0 if blk == 0 else S1
                last = n == NE - 1
                for gi, (g0, g1) in enumerate(L_GROUPS):
                    gl = g1 - g0
                    apool, atag = ((psarc, "arc") if (n + gi) % 2 == 0
                                   else (psmm, "pmm"))
                    arc = apool.tile([128, 512], F32, tag=atag, name="arc")
                    for l in range(g0, g1):
                        off = (l - g0) * N1
                        for h, psrc in ((0, ph), (1, pl)):
                            for a in range(KT // 2):
                                nc.tensor.matmul(
                                    arc[:, off:off + N1],
                                    am_pair(am_res, n, a, l),
                                    p_pair(psrc, a, n),
                                    start=(h == 0 and a == 0),
                                    stop=(h == 1 and a == KT // 2 - 1),
                                    perf_mode=DR)
                    g_sb = gpool.tile([128, 4], F32, tag="g")
                    nc.scalar.activation(
                        g_sb[:, 0:gl], arc[:, D:D + (gl - 1) * N1 + 1:N1],
                        mybir.ActivationFunctionType.Sigmoid, scale=sgate)
                    u = 3 * n + gi
                    if u >= 18:
                        # the late types' arcs finish at the very end: strict
                        # A/B alternation parallelizes their gating tail
                        path_b = u % 2 == 0
                    else:
                        path_b = u % 9 < 4
                    # bf16 staging tile the block-0 final accumulate lands in
                    if last and blk == 0:
                        accb = gpool.tile([128, 3 * D], BF16, tag="accb",
                                          bufs=3, name="accb")

                    def gout(l):
                        if not last:
                            return accA[:, l * D:(l + 1) * D]
                        if blk == 0:
                            return accb[:, (l - g0) * D:(l - g0 + 1) * D]
                        return accA[:, l * D:(l + 1) * D]

                    if path_b:
                        for l in range(g0, g1):
                            off = (l - g0) * N1
                            t = gpool.tile([128, D], F32, tag="gt", bufs=6,
                                           name="gt")
                            nc.scalar.activation(
                                t[:], arc[:, off:off + D],
                                mybir.ActivationFunctionType.Copy,
                                scale=g_sb[:, l - g0:l - g0 + 1])
                            nc.gpsimd.tensor_tensor(
                                out=gout(l),
                                in0=accA[:, l * D:(l + 1) * D],
                                in1=t[:], op=mybir.AluOpType.add)
                    else:
                        for l in range(g0, g1):
                            off = (l - g0) * N1
                            nc.vector.scalar_tensor_tensor(
                                out=gout(l),
                                in0=arc[:, off:off + D],
                                scalar=g_sb[:, l - g0:l - g0 + 1],
                                in1=accA[:, l * D:(l + 1) * D],
                                op0=mybir.AluOpType.mult,
                                op1=mybir.AluOpType.add)
                    if last and blk == 0:
                        nc.sync.dma_start(
                            cc_in.ap()[g0 * 128:g1 * 128, :]
                            .rearrange("(t p) d -> p t d", p=128),
                            accb[:, 0:gl * D].rearrange(
                                "p (t d) -> p t d", d=D))
                    elif last:
                        nc.sync.dma_start(
                            outp_d.ap()[rep, g0 * 128:g1 * 128, :]
                            .rearrange("(t p) d -> p t d", p=128),
                            accA[:, g0 * D:g1 * D].rearrange(
                                "p (t d) -> p t d", d=D))

            for rep in range(reps):
                # ---- first p-matmul deps lead each queue (xt chunk 0 on
                # sync, w0 chunk 0 on scalar), then the adjacency stream owns
                # sync while the remaining small inputs trickle on scalar ----
                xt = xp.tile([128, 2 * L], BF16, tag="xt")
                w0 = [wp.tile([E0, NW], BF16, tag="w0", name=f"w0_{i}")
                      for i in range(NB)]
                w1 = [wp.tile([16, NW], BF16, tag="w1", name=f"w1_{i}")
                      for i in range(NB)]
                if CFG["early_q"] == "split":
                    nc.sync.dma_start(xt[:, 0:256], xt0a_d.ap()[:, 0:256])
                    nc.scalar.dma_start(w0[0][:, 0:512], w0_d.ap()[0][:, 0:512])
                    am_res = [[load_am(n, half) for half in range(KT // KS)]
                              for n in range(NE)]
                    for q in range(1, 4):
                        nc.scalar.dma_start(xt[:, q * 256:(q + 1) * 256],
                                            xt0a_d.ap()[:, q * 256:(q + 1) * 256])
                    nc.scalar.dma_start(xt[0:16, L:2 * L], xt0b_d.ap())
                    for (c0, c1) in P_CHUNKS[1:]:
                        nc.scalar.dma_start(w0[0][:, c0:c1],
                                            w0_d.ap()[0][:, c0:c1])
                    nc.scalar.dma_start(w1[0][:], w1_d.ap()[0])
                else:
                    # first m-tile chain needs xt q0 + xtb + w0 c0 + w1: put
                    # the tiny tail slabs right after the heads of each queue
                    nc.sync.dma_start(xt[:, 0:256], xt0a_d.ap()[:, 0:256])
                    nc.sync.dma_start(xt[0:16, L:2 * L], xt0b_d.ap())
                    for q in range(1, 4):
                        nc.sync.dma_start(xt[:, q * 256:(q + 1) * 256],
                                          xt0a_d.ap()[:, q * 256:(q + 1) * 256])
                    nc.scalar.dma_start(w0[0][:, 0:512], w0_d.ap()[0][:, 0:512])
                    nc.scalar.dma_start(w1[0][:], w1_d.ap()[0])
                    for (c0, c1) in P_CHUNKS[1:]:
                        nc.scalar.dma_start(w0[0][:, c0:c1],
                                            w0_d.ap()[0][:, c0:c1])
                    am_res = [[load_am(n, half) for half in range(KT // KS)]
                              for n in range(NE)]

                # residual stream + block-1 weights (needed much later)
                xf = xp.tile([128, LT * E], F32, tag="xf")
                xf_eng = nc.sync if CFG["xf_q"] == "sync_late" else nc.gpsimd
                xf_eng.dma_start(xf[:].rearrange("p (t d) -> p t d", t=LT),
                                 x0_d.ap().rearrange("(t p) d -> p t d", p=128))
                xf_eng.dma_start(w0[1][:], w0_d.ap()[1])
                xf_eng.dma_start(w1[1][:], w1_d.ap()[1])

                # ---- block 0: p-chunks interleaved with arc types so the PE
                # stays busy while the adjacency streams in ----
                acc = xp.tile([128, LT * D], F32, tag="acc")
                accB = xp.tile([128, LT * D], F32, tag="accB")
                nc.gpsimd.memset(acc[:], 0.0)
                nc.gpsimd.memset(accB[:], 0.0)
                ph = [ppool.tile([128, 2 * NW], FP8, tag="ph",
                                 name=f"ph_0_{a}") for a in range(4)]
                pl = [ppool.tile([128, 2 * NW], FP8, tag="pl",
                                 name=f"pl_0_{a}") for a in range(4)]
                # both p-chunks first (PE fills the DMA-latency window and
                # the c0 lo-quantizes drain on DVE under the c1 matmuls)
                emit_p(0, w0, w1, xt, ph, pl, range(KT), corder=True, cs=[0])
                if CFG["pboth"]:
                    emit_p(0, w0, w1, xt, ph, pl, range(KT), corder=True,
                           cs=[1])
                    for n in range(NE):
                        emit_arc(0, n, am_res, ph, pl, acc, accB, rep)
                else:
                    emit_arc(0, 0, am_res, ph, pl, acc, accB, rep)
                    emit_arc(0, 1, am_res, ph, pl, acc, accB, rep)
                    emit_p(0, w0, w1, xt, ph, pl, range(KT), corder=True,
                           cs=[1])
                    for n in range(2, NE):
                        emit_arc(0, n, am_res, ph, pl, acc, accB, rep)

                # single pair AllGather of the full bf16 partial; each core
                # then sums both halves locally (one fixed overhead, no
                # AllReduce 1.875x factor)
                nc.gpsimd.collective_compute(
                    "AllGather", mybir.AluOpType.bypass,
                    replica_groups=PAIRS,
                    ins=[cc_in.ap()], outs=[cc_out.ap()])

                # ---- post-AG chain ----
                # x1b layout: 256 cols per l-tile (140 data + ones col + junk)
                # so one DMA xbar transpose per tile lands both the main
                # 128-row slab and the 13-row tail of x~1^T
                x1 = xp.tile([128, LT * E], F32, tag="xf")
                x1b = xp.tile([128, LT * 256], BF16, tag="x1b")
                x1b3 = x1b[:].rearrange("p (t c) -> p t c", c=256)
                nc.gpsimd.memset(x1b3[:, :, 140:256], 1.0)
                xt_n = xp.tile([128, 2 * L], BF16, tag="xt")
                red = xp.tile([128, LT * E], BF16, tag="red")
                redb = xp.tile([128, LT * E], BF16, tag="redb")
                ph_next = [ppool.tile([128, 2 * NW], FP8, tag="ph",
                                      name=f"ph_1_{a}") for a in range(4)]
                pl_next = [ppool.tile([128, 2 * NW], FP8, tag="pl",
                                      name=f"pl_1_{a}") for a in range(4)]
                xf3 = xf[:].rearrange("p (t c) -> p t c", c=E)
                # red/redb first (they only depend on the AG) so no other
                # queue work head-of-line blocks them; tile 0 alone first so
                # the first transpose + p-matmuls start ASAP
                SEGS = ([(0, 1), (1, 3), (3, 6), (6, 8)]
                        if CFG["segs"] == "fine" else list(L_GROUPS))
                for (g0, g1) in SEGS:
                    sl = slice(g0 * E, g1 * E)
                    nc.sync.dma_start(
                        red[:, sl].rearrange("p (t d) -> p t d", d=E),
                        cc_out.ap()[g0 * 128:g1 * 128, :].rearrange(
                            "(t p) d -> p t d", p=128))
                    nc.gpsimd.dma_start(
                        redb[:, sl].rearrange("p (t d) -> p t d", d=E),
                        cc_out.ap()[L + g0 * 128:L + g1 * 128, :]
                        .rearrange("(t p) d -> p t d", p=128))
                for (g0, g1) in SEGS:
                    sl = slice(g0 * E, g1 * E)
                    nc.vector.tensor_tensor(
                        out=x1[:, sl], in0=red[:, sl], in1=redb[:, sl],
                        op=mybir.AluOpType.add)
                    nc.vector.scalar_tensor_tensor(
                        out=x1b3[:, g0:g1, 0:E], in0=x1[:, sl], scalar=0.0,
                        in1=xf3[:, g0:g1, :], op0=mybir.AluOpType.max,
                        op1=mybir.AluOpType.add)
                    nc.gpsimd.dma_start(
                        x1o_d.ap()[rep, g0 * 128:g1 * 128, :].rearrange(
                            "(t p) d -> p t d", p=128),
                        x1b3[:, g0:g1, 0:E])
                    for lt in range(g0, g1):
                        # one xbar transpose: cols 0:128 -> x~1^T main slab,
                        # cols 128:256 -> tail slab (rows 13.. are junk that
                        # the 13-row contraction never reads); alternate the
                        # issue queue so issue overhead doesn't starve the PE
                        teng = (nc.sync if lt % 2 == 0 or not CFG["t_alt"]
                                else nc.scalar)
                        teng.dma_start_transpose(
                            xt_n[:].rearrange("p (j c) -> p j c", c=L)[
                                :, :, lt * 128:(lt + 1) * 128],
                            x1b[:, lt * 256:(lt + 1) * 256])
                        emit_p(1, w0, w1, xt_n, ph_next, pl_next, [lt],
                               corder=False)

                # ---- block 1 arcs ----
                acc1 = xp.tile([128, LT * D], F32, tag="acc")
                accB1 = xp.tile([128, LT * D], F32, tag="accB")
                nc.gpsimd.memset(acc1[:], 0.0)
                nc.gpsimd.memset(accB1[:], 0.0)
                for n in range(NE):
                    emit_arc(1, n, am_res, ph_next, pl_next, acc1, accB1, rep)

    nc.compile()
    return nc


def _get_nc():
    global _NC
    if _NC is None:
        _NC = _build()
    return _NC


def _prep_inputs(seq_repr, adj, W_in, b_in, W_out, b_out,
                 Wg_in, bg_in, Wg_out, bg_out):
    """Build the 8 per-core input maps (host-side sharding + layout prep)."""
    et = np.minimum(np.arange(NE), NU)
    seq_repr = np.asarray(seq_repr, np.float32)
    adj = np.asarray(adj)

    # x~0^T slabs, shared by all cores of the same b
    xt_by_b = []
    for b in range(B):
        xt = np.concatenate(
            [seq_repr[b], np.ones((L, 1), np.float32)], axis=1).T  # (141, L)
        xt = xt.astype(BF)
        xt0b = np.zeros((16, L), BF)
        xt0b[0:E1] = xt[E0:E + 1]
        xt_by_b.append((np.ascontiguousarray(xt[0:E0]), xt0b))

    # weight slabs per direction: rows = e (140) + bias row; cols = ND*(D+1)
    # (only the ND distinct weight sets -- shared types reuse set NU)
    def wslabs(Wd, bd, Wgd, bgd):
        w = np.zeros((NB, E + 1, NW), np.float32)
        for blk in range(NB):
            for s in range(ND):
                w[blk, 0:E, s * N1:s * N1 + D] = Wd[blk, s]
                w[blk, E, s * N1:s * N1 + D] = bd[blk, s]
                w[blk, 0:E, s * N1 + D] = Wgd[blk, s, :, 0]
                w[blk, E, s * N1 + D] = bgd[blk, s, 0]
        w = w.astype(BF)
        w1 = np.zeros((NB, 16, NW), BF)
        w1[:, 0:E1] = w[:, E0:E + 1]
        return np.ascontiguousarray(w[:, 0:E0]), w1

    w_in0, w_in1 = wslabs(np.asarray(W_in, np.float32), np.asarray(b_in, np.float32),
                          np.asarray(Wg_in, np.float32), np.asarray(bg_in, np.float32))
    w_out0, w_out1 = wslabs(np.asarray(W_out, np.float32), np.asarray(b_out, np.float32),
                            np.asarray(Wg_out, np.float32), np.asarray(bg_out, np.float32))

    in_maps = []
    for c in range(NCORES):
        b, dirn = c // 2, c % 2
        a = adj[b]  # (NE, L, L) int32
        if dirn == 0:
            # in-arcs: lhsT tile [m, l] must hold A[l, m] -> transpose
            am = np.ascontiguousarray(a.transpose(0, 2, 1)).astype(E4)
            w0, w1 = w_in0, w_in1
        else:
            am = np.ascontiguousarray(a).astype(E4)
            w0, w1 = w_out0, w_out1
        xt0a, xt0b = xt_by_b[b]
        in_maps.append({
            "am": am, "x0": np.ascontiguousarray(seq_repr[b]),
            "xt0a": xt0a, "xt0b": xt0b, "w0": w0, "w1": w1,
        })
    return in_maps


def _combine(results):
    """Host epilogue: x2 = relu(S1*(p_in + p_out)) + x1 per batch."""
    out = np.empty((B, L, E), np.float32)
    for b in range(B):
        pin = results[2 * b]["outp"][0]
        pout = results[2 * b + 1]["outp"][0]
        x1 = results[2 * b]["x1o"][0].astype(np.float32)
        out[b] = np.maximum((pin + pout) * S1, 0.0) + x1
    return out


def run_on_hw(in_maps, trace=False, **kw):
    nc = _get_nc()
    res = run_bass_kernel_spmd(nc, in_maps, core_ids=list(range(NCORES)),
                               trace=trace, **kw)
    return res


def kernel(**inputs):
    in_maps = _prep_inputs(**inputs)
    res = run_on_hw(in_maps)
    return _combine(res.results)


# revision 55
# speedup vs baseline: 3.1791x; 1.1008x over previous
"""Bass/Trainium2 kernel for nn_GCNN_61615600828570 (gated GCNN message passing).

Self-contained: hardcodes shapes/sharding. 8 NeuronCores, sharded as
(batch b, arc-direction) pairs; single pair AllGather between the two GCN
blocks (each core then sums both partials locally).

Arc aggregation runs in fp8 (float8e4) DoubleRow perf mode at 2x PE rate:
the 0/1 adjacency is exact in fp8, and the per-type projections p are split
hi+lo (p ~= fp8(p) + fp8(p - fp8(p))), giving bf16-level accuracy at fp8
speed. Block-1 projections are pre-scaled by 1/8 to stay inside e4m3 range
(max 240); the scale is undone in the gate sigmoid (scale=8) and on the
host for the block-1 output partials.

kernel(**inputs) takes the FULL inputs (numpy, dtypes as in setup_inputs)
and returns the FULL (B, L, E) float32 output.
"""
import numpy as np
import ml_dtypes

import concourse.bass as bass
import concourse.mybir as mybir
import concourse.tile as tile
from concourse import bacc
from concourse.bass_utils import run_bass_kernel_spmd

F32 = mybir.dt.float32
BF16 = mybir.dt.bfloat16
FP8 = mybir.dt.float8e4
BF = ml_dtypes.bfloat16
E4 = ml_dtypes.float8_e4m3

B, L, E, D = 4, 1024, 140, 140
NE, NU, NB = 10, 4, 2
ND = NU + 1
N1 = D + 1            # 141: D outputs + gate column
LT = L // 128         # 8 l-tiles
KT = L // 128         # 8 contraction tiles for arc
E0 = 128              # first x~ k-slab rows
E1 = E + 1 - E0       # 13: remaining e rows + ones row
NCORES = 8
PAIRS = [[0, 1], [2, 3], [4, 5], [6, 7]]
NW = ND * N1          # 705: distinct projection columns (types >= NU share
                      # weight set NU, so only ND=5 column groups are needed)
P_CHUNKS = [(0, 512), (512, NW)]  # psum-bank sized N-chunks
L_GROUPS = [(0, 3), (3, 6), (6, 8)]  # l-tile groups per arc psum bank
KS = 4                # k-slabs per adjacency super-tile (1 MB DMAs)
S1 = 8.0              # block-1 fp8 pre-scale (keeps |p|/S1 << e4m3 max 240)
DR = mybir.MatmulPerfMode.DoubleRow

_NC = None


# build-time scheduling knobs (tuned against TimelineSim)
CFG = {
    "early_q": "scalar",   # "split": xt q0 sync + rest scalar; "scalar": all sync/scalar
    "xf_q": "sync_late",   # "sync_late" | "gpsimd"
    "interleave": "1x1",   # "2x2": c0 a0 a1 c1 a2 a3 c2 | "1x1": c0 a0 c1 a1 a2 c2
    "par_flip": False,     # flip gating parity for last two types
    "segs": "groups",      # "fine": (0,1)(1,3)(3,6)(6,8) | "groups"
    "t_alt": True,         # alternate transpose queue sync/scalar
    "outp_tile": False,    # per-l-tile outp DMA for last group
    "tail": "g1b",         # (unused with split accumulators)
    "pboth": True,         # emit both block-0 p-chunks before the arcs
    "bsel": "9_4",         # B-path unit selector: 9_4 | 5_3 | 3_1
}
# edge types whose gating goes through the SBUF-staged gpsimd path
PB_TYPES = {1, 3, 5, 7, 8}


def _build(reps=1):
    nc = bacc.Bacc("TRN2", target_bir_lowering=False, debug=False,
                   num_devices=NCORES)

    am_d = nc.dram_tensor("am", [NE, L, L], FP8, kind="ExternalInput")
    x0_d = nc.dram_tensor("x0", [L, E], F32, kind="ExternalInput")
    xt0a_d = nc.dram_tensor("xt0a", [E0, L], BF16, kind="ExternalInput")
    xt0b_d = nc.dram_tensor("xt0b", [16, L], BF16, kind="ExternalInput")
    w0_d = nc.dram_tensor("w0", [NB, E0, NW], BF16, kind="ExternalInput")
    w1_d = nc.dram_tensor("w1", [NB, 16, NW], BF16, kind="ExternalInput")

    outp_d = nc.dram_tensor("outp", [reps, L, E], F32, kind="ExternalOutput")
    x1o_d = nc.dram_tensor("x1o", [reps, L, E], BF16, kind="ExternalOutput")

    cc_in = nc.dram_tensor("cc_in", [L, E], BF16)
    cc_out = nc.dram_tensor("cc_out", [2 * L, E], BF16)

    with tile.TileContext(nc) as tc:
        with (
            tc.tile_pool(name="amr", bufs=NE * (KT // KS)) as amr,
            tc.tile_pool(name="pp", bufs=KT // 2) as ppool,
            tc.tile_pool(name="wp", bufs=2) as wp,
            tc.tile_pool(name="xp", bufs=2) as xp,
            tc.tile_pool(name="gp", bufs=8) as gpool,
            tc.tile_pool(name="psarc", bufs=4, space="PSUM") as psarc,
            tc.tile_pool(name="psmm", bufs=4, space="PSUM") as psmm,
        ):
            am_view = am_d.ap().rearrange("n (j p) c -> n p j c", p=128)

            def load_am(n, half):
                t = amr.tile([128, KS * L], FP8, tag="amr",
                             name=f"amr_{n}_{half}")
                nc.sync.dma_start(
                    t[:].rearrange("p (j c) -> p j c", c=L),
                    am_view[n, :, half * KS:(half + 1) * KS, :])
                return t

            def am_pair(am_res, n, a, l):
                # lhsT AP [128, 2, 128]: k-slab pair (2a, 2a+1), l-tile cols
                t = am_res[n][a // 2]
                return t[:].rearrange("p (j c) -> p j c", c=L)[
                    :, 2 * (a % 2):2 * (a % 2) + 2, l * 128:(l + 1) * 128]

            def p_pair(psrc, a, n):
                # rhs AP [128, 2, N1]: k-slab pair (2a, 2a+1), cols of the
                # weight set shared by type n
                s = min(n, NU)
                return psrc[a][:].rearrange("p (j c) -> p j c", c=NW)[
                    :, :, s * N1:(s + 1) * N1]

            def emit_p(blk, w0, w1, xt_ap, ph, pl, ms, corder, cs=None):
                if cs is None:
                    cs = range(len(P_CHUNKS))
                inv_s = 1.0 if blk == 0 else 1.0 / S1
                seq = ([(c, m) for c in cs for m in ms] if corder
                       else [(c, m) for m in ms for c in cs])
                for (c, m) in seq:
                    c0, c1 = P_CHUNKS[c]
                    mpool = psmm if m % 2 == 0 else psarc
                    mtag = "pmm" if m % 2 == 0 else "arc"
                    pmm = mpool.tile([128, 512], F32, tag=mtag, name="pmm")
                    nc.tensor.matmul(
                        pmm[:, 0:c1 - c0],
                        xt_ap[:, m * 128:(m + 1) * 128],
                        w0[blk][:, c0:c1], start=True, stop=False)
                    nc.tensor.matmul(
                        pmm[:, 0:c1 - c0],
                        xt_ap[0:E1, L + m * 128:L + (m + 1) * 128],
                        w1[blk][0:E1, c0:c1], start=False, stop=True)
                    po = (m % 2) * NW
                    hi = ph[m // 2][:, po + c0:po + c1]
                    if blk == 0:
                        nc.scalar.copy(hi, pmm[:, 0:c1 - c0])
                    else:
                        nc.scalar.activation(
                            hi, pmm[:, 0:c1 - c0],
                            mybir.ActivationFunctionType.Copy, scale=inv_s)
                    # lo-residual quantize (DVE: gpsimd cannot access PSUM)
                    nc.vector.scalar_tensor_tensor(
                        out=pl[m // 2][:, po + c0:po + c1],
                        in0=pmm[:, 0:c1 - c0], scalar=inv_s, in1=hi,
                        op0=mybir.AluOpType.mult,
                        op1=mybir.AluOpType.subtract)

            def emit_arc(blk, n, am_res, ph, pl, accA, accB, rep):
                """All l-groups of one edge type: DoubleRow hi+lo chains,
                gate sigmoid, gated accumulate. Two independent accumulator
                chains (A: DVE straight from psum; B: Act gate-scaled copy +
                Pool add from SBUF) merged per group after the last type."""
                sgate = 1.0 if blk == 0 else S1
                last = n == NE - 1
                for gi, (g0, g1) in enumerate(L_GROUPS):
                    gl = g1 - g0
                    apool, atag = ((psarc, "arc") if (n + gi) % 2 == 0
                                   else (psmm, "pmm"))
                    arc = apool.tile([128, 512], F32, tag=atag, name="arc")
                    for l in range(g0, g1):
                        off = (l - g0) * N1
                        for h, psrc in ((0, ph), (1, pl)):
                            for a in range(KT // 2):
                                nc.tensor.matmul(
                                    arc[:, off:off + N1],
                                    am_pair(am_res, n, a, l),
                                    p_pair(psrc, a, n),
                                    start=(h == 0 and a == 0),
                                    stop=(h == 1 and a == KT // 2 - 1),
                                    perf_mode=DR)
                    g_sb = gpool.tile([128, 4], F32, tag="g")
                    nc.scalar.activation(
                        g_sb[:, 0:gl], arc[:, D:D + (gl - 1) * N1 + 1:N1],
                        mybir.ActivationFunctionType.Sigmoid, scale=sgate)
                    u = 3 * n + gi
                    if u >= 18:
                        # the late types' arcs finish at the very end: strict
                        # A/B alternation parallelizes their gating tail
                        path_b = u % 2 == 0
                    else:
                        path_b = u % 9 < 4
                    # bf16 staging tile the block-0 final accumulate lands in
                    if last and blk == 0:
                        accb = gpool.tile([128, 3 * D], BF16, tag="accb",
                                          bufs=3, name="accb")

                    def gout(l):
                        if not last:
                            return accA[:, l * D:(l + 1) * D]
                        if blk == 0:
                            return accb[:, (l - g0) * D:(l - g0 + 1) * D]
                        return accA[:, l * D:(l + 1) * D]

                    if path_b:
                        for l in range(g0, g1):
                            off = (l - g0) * N1
                            t = gpool.tile([128, D], F32, tag="gt", bufs=6,
                                           name="gt")
                            nc.scalar.activation(
                                t[:], arc[:, off:off + D],
                                mybir.ActivationFunctionType.Copy,
                                scale=g_sb[:, l - g0:l - g0 + 1])
                            nc.gpsimd.tensor_tensor(
                                out=gout(l),
                                in0=accA[:, l * D:(l + 1) * D],
                                in1=t[:], op=mybir.AluOpType.add)
                    else:
                        for l in range(g0, g1):
                            off = (l - g0) * N1
                            nc.vector.scalar_tensor_tensor(
                                out=gout(l),
                                in0=arc[:, off:off + D],
                                scalar=g_sb[:, l - g0:l - g0 + 1],
                                in1=accA[:, l * D:(l + 1) * D],
                                op0=mybir.AluOpType.mult,
                                op1=mybir.AluOpType.add)
                    if last and blk == 0:
                        nc.sync.dma_start(
                            cc_in.ap()[g0 * 128:g1 * 128, :]
                            .rearrange("(t p) d -> p t d", p=128),
                            accb[:, 0:gl * D].rearrange(
                                "p (t d) -> p t d", d=D))
                    elif last:
                        nc.sync.dma_start(
                            outp_d.ap()[rep, g0 * 128:g1 * 128, :]
                            .rearrange("(t p) d -> p t d", p=128),
                            accA[:, g0 * D:g1 * D].rearrange(
                                "p (t d) -> p t d", d=D))

            for rep in range(reps):
                # ---- first p-matmul deps lead each queue (xt chunk 0 on
                # sync, w0 chunk 0 on scalar), then the adjacency stream owns
                # sync while the remaining small inputs trickle on scalar ----
                xt = xp.tile([128, 2 * L], BF16, tag="xt")
                w0 = [wp.tile([E0, NW], BF16, tag="w0", name=f"w0_{i}")
                      for i in range(NB)]
                w1 = [wp.tile([16, NW], BF16, tag="w1", name=f"w1_{i}")
                      for i in range(NB)]
                if CFG["early_q"] == "split":
                    nc.sync.dma_start(xt[:, 0:256], xt0a_d.ap()[:, 0:256])
                    nc.scalar.dma_start(w0[0][:, 0:512], w0_d.ap()[0][:, 0:512])
                    am_res = [[load_am(n, half) for half in range(KT // KS)]
                              for n in range(NE)]
                    for q in range(1, 4):
                        nc.scalar.dma_start(xt[:, q * 256:(q + 1) * 256],
                                            xt0a_d.ap()[:, q * 256:(q + 1) * 256])
                    nc.scalar.dma_start(xt[0:16, L:2 * L], xt0b_d.ap())
                    for (c0, c1) in P_CHUNKS[1:]:
                        nc.scalar.dma_start(w0[0][:, c0:c1],
                                            w0_d.ap()[0][:, c0:c1])
                    nc.scalar.dma_start(w1[0][:], w1_d.ap()[0])
                else:
                    # first m-tile chain needs xt q0 + xtb + w0 c0 + w1:
                    # spread those four over three queues so none serializes
                    nc.sync.dma_start(xt[:, 0:256], xt0a_d.ap()[:, 0:256])
                    nc.scalar.dma_start(w0[0][:, 0:512], w0_d.ap()[0][:, 0:512])
                    nc.gpsimd.dma_start(xt[0:16, L:2 * L], xt0b_d.ap())
                    nc.gpsimd.dma_start(w1[0][:], w1_d.ap()[0])
                    for q in range(1, 4):
                        nc.sync.dma_start(xt[:, q * 256:(q + 1) * 256],
                                          xt0a_d.ap()[:, q * 256:(q + 1) * 256])
                    for (c0, c1) in P_CHUNKS[1:]:
                        nc.scalar.dma_start(w0[0][:, c0:c1],
                                            w0_d.ap()[0][:, c0:c1])
                    am_res = [[load_am(n, half) for half in range(KT // KS)]
                              for n in range(NE)]

                # residual stream + block-1 weights (needed much later)
                xf = xp.tile([128, LT * E], F32, tag="xf")
                xf_eng = nc.sync if CFG["xf_q"] == "sync_late" else nc.gpsimd
                xf_eng.dma_start(xf[:].rearrange("p (t d) -> p t d", t=LT),
                                 x0_d.ap().rearrange("(t p) d -> p t d", p=128))
                xf_eng.dma_start(w0[1][:], w0_d.ap()[1])
                xf_eng.dma_start(w1[1][:], w1_d.ap()[1])

                # ---- block 0: p-chunks interleaved with arc types so the PE
                # stays busy while the adjacency streams in ----
                acc = xp.tile([128, LT * D], F32, tag="acc")
                accB = xp.tile([128, LT * D], F32, tag="accB")
                nc.gpsimd.memset(acc[:], 0.0)
                nc.gpsimd.memset(accB[:], 0.0)
                ph = [ppool.tile([128, 2 * NW], FP8, tag="ph",
                                 name=f"ph_0_{a}") for a in range(4)]
                pl = [ppool.tile([128, 2 * NW], FP8, tag="pl",
                                 name=f"pl_0_{a}") for a in range(4)]
                # both p-chunks first (PE fills the DMA-latency window and
                # the c0 lo-quantizes drain on DVE under the c1 matmuls)
                emit_p(0, w0, w1, xt, ph, pl, range(KT), corder=True, cs=[0])
                if CFG["pboth"]:
                    emit_p(0, w0, w1, xt, ph, pl, range(KT), corder=True,
                           cs=[1])
                    for n in range(NE):
                        emit_arc(0, n, am_res, ph, pl, acc, accB, rep)
                else:
                    emit_arc(0, 0, am_res, ph, pl, acc, accB, rep)
                    emit_arc(0, 1, am_res, ph, pl, acc, accB, rep)
                    emit_p(0, w0, w1, xt, ph, pl, range(KT), corder=True,
                           cs=[1])
                    for n in range(2, NE):
                        emit_arc(0, n, am_res, ph, pl, acc, accB, rep)

                # single pair AllGather of the full bf16 partial; each core
                # then sums both halves locally (one fixed overhead, no
                # AllReduce 1.875x factor)
                nc.gpsimd.collective_compute(
                    "AllGather", mybir.AluOpType.bypass,
                    replica_groups=PAIRS,
                    ins=[cc_in.ap()], outs=[cc_out.ap()])

                # ---- post-AG chain ----
                # x1b layout: 256 cols per l-tile (140 data + ones col + junk)
                # so one DMA xbar transpose per tile lands both the main
                # 128-row slab and the 13-row tail of x~1^T
                x1 = xp.tile([128, LT * E], F32, tag="xf")
                x1b = xp.tile([128, LT * 256], BF16, tag="x1b")
                x1b3 = x1b[:].rearrange("p (t c) -> p t c", c=256)
                nc.gpsimd.memset(x1b3[:, :, 140:256], 1.0)
                xt_n = xp.tile([128, 2 * L], BF16, tag="xt")
                red = xp.tile([128, LT * E], BF16, tag="red")
                redb = xp.tile([128, LT * E], BF16, tag="redb")
                ph_next = [ppool.tile([128, 2 * NW], FP8, tag="ph",
                                      name=f"ph_1_{a}") for a in range(4)]
                pl_next = [ppool.tile([128, 2 * NW], FP8, tag="pl",
                                      name=f"pl_1_{a}") for a in range(4)]
                xf3 = xf[:].rearrange("p (t c) -> p t c", c=E)
                # red/redb first (they only depend on the AG) so no other
                # queue work head-of-line blocks them; tile 0 alone first so
                # the first transpose + p-matmuls start ASAP
                SEGS = ([(0, 1), (1, 3), (3, 6), (6, 8)]
                        if CFG["segs"] == "fine" else list(L_GROUPS))
                for (g0, g1) in SEGS:
                    sl = slice(g0 * E, g1 * E)
                    nc.sync.dma_start(
                        red[:, sl].rearrange("p (t d) -> p t d", d=E),
                        cc_out.ap()[g0 * 128:g1 * 128, :].rearrange(
                            "(t p) d -> p t d", p=128))
                    nc.gpsimd.dma_start(
                        redb[:, sl].rearrange("p (t d) -> p t d", d=E),
                        cc_out.ap()[L + g0 * 128:L + g1 * 128, :]
                        .rearrange("(t p) d -> p t d", p=128))
                for (g0, g1) in SEGS:
                    sl = slice(g0 * E, g1 * E)
                    nc.vector.tensor_tensor(
                        out=x1[:, sl], in0=red[:, sl], in1=redb[:, sl],
                        op=mybir.AluOpType.add)
                    nc.vector.scalar_tensor_tensor(
                        out=x1b3[:, g0:g1, 0:E], in0=x1[:, sl], scalar=0.0,
                        in1=xf3[:, g0:g1, :], op0=mybir.AluOpType.max,
                        op1=mybir.AluOpType.add)
                    nc.gpsimd.dma_start(
                        x1o_d.ap()[rep, g0 * 128:g1 * 128, :].rearrange(
                            "(t p) d -> p t d", p=128),
                        x1b3[:, g0:g1, 0:E])
                    for lt in range(g0, g1):
                        # one xbar transpose: cols 0:128 -> x~1^T main slab,
                        # cols 128:256 -> tail slab (rows 13.. are junk that
                        # the 13-row contraction never reads); alternate the
                        # issue queue so issue overhead doesn't starve the PE
                        teng = (nc.sync if lt % 2 == 0 or not CFG["t_alt"]
                                else nc.scalar)
                        teng.dma_start_transpose(
                            xt_n[:].rearrange("p (j c) -> p j c", c=L)[
                                :, :, lt * 128:(lt + 1) * 128],
                            x1b[:, lt * 256:(lt + 1) * 256])
                        emit_p(1, w0, w1, xt_n, ph_next, pl_next, [lt],
                               corder=False)

                # ---- block 1 arcs ----
                acc1 = xp.tile([128, LT * D], F32, tag="acc")
                accB1 = xp.tile([128, LT * D], F32, tag="accB")
                nc.gpsimd.memset(acc1[:], 0.0)
                nc.gpsimd.memset(accB1[:], 0.0)
                for n in range(NE):
                    emit_arc(1, n, am_res, ph_next, pl_next, acc1, accB1, rep)

    nc.compile()
    return nc


def _get_nc():
    global _NC
    if _NC is None:
        _NC = _build()
    return _NC


def _prep_inputs(seq_repr, adj, W_in, b_in, W_out, b_out,
                 Wg_in, bg_in, Wg_out, bg_out):
    """Build the 8 per-core input maps (host-side sharding + layout prep)."""
    et = np.minimum(np.arange(NE), NU)
    seq_repr = np.asarray(seq_repr, np.float32)
    adj = np.asarray(adj)

    # x~0^T slabs, shared by all cores of the same b
    xt_by_b = []
    for b in range(B):
        xt = np.concatenate(
            [seq_repr[b], np.ones((L, 1), np.float32)], axis=1).T  # (141, L)
        xt = xt.astype(BF)
        xt0b = np.zeros((16, L), BF)
        xt0b[0:E1] = xt[E0:E + 1]
        xt_by_b.append((np.ascontiguousarray(xt[0:E0]), xt0b))

    # weight slabs per direction: rows = e (140) + bias row; cols = ND*(D+1)
    # (only the ND distinct weight sets -- shared types reuse set NU)
    def wslabs(Wd, bd, Wgd, bgd):
        w = np.zeros((NB, E + 1, NW), np.float32)
        for blk in range(NB):
            for s in range(ND):
                w[blk, 0:E, s * N1:s * N1 + D] = Wd[blk, s]
                w[blk, E, s * N1:s * N1 + D] = bd[blk, s]
                w[blk, 0:E, s * N1 + D] = Wgd[blk, s, :, 0]
                w[blk, E, s * N1 + D] = bgd[blk, s, 0]
        w = w.astype(BF)
        w1 = np.zeros((NB, 16, NW), BF)
        w1[:, 0:E1] = w[:, E0:E + 1]
        return np.ascontiguousarray(w[:, 0:E0]), w1

    w_in0, w_in1 = wslabs(np.asarray(W_in, np.float32), np.asarray(b_in, np.float32),
                          np.asarray(Wg_in, np.float32), np.asarray(bg_in, np.float32))
    w_out0, w_out1 = wslabs(np.asarray(W_out, np.float32), np.asarray(b_out, np.float32),
                            np.asarray(Wg_out, np.float32), np.asarray(bg_out, np.float32))

    in_maps = []
    for c in range(NCORES):
        b, dirn = c // 2, c % 2
        a = adj[b]  # (NE, L, L) int32
        if dirn == 0:
            # in-arcs: lhsT tile [m, l] must hold A[l, m] -> transpose
            am = np.ascontiguousarray(a.transpose(0, 2, 1)).astype(E4)
            w0, w1 = w_in0, w_in1
        else:
            am = np.ascontiguousarray(a).astype(E4)
            w0, w1 = w_out0, w_out1
        xt0a, xt0b = xt_by_b[b]
        in_maps.append({
            "am": am, "x0": np.ascontiguousarray(seq_repr[b]),
            "xt0a": xt0a, "xt0b": xt0b, "w0": w0, "w1": w1,
        })
    return in_maps


def _combine(results):
    """Host epilogue: x2 = relu(S1*(p_in + p_out)) + x1 per batch."""
    out = np.empty((B, L, E), np.float32)
    for b in range(B):
        pin = results[2 * b]["outp"][0]
        pout = results[2 * b + 1]["outp"][0]
        x1 = results[2 * b]["x1o"][0].astype(np.float32)
        out[b] = np.maximum((pin + pout) * S1, 0.0) + x1
    return out


def run_on_hw(in_maps, trace=False, **kw):
    nc = _get_nc()
    res = run_bass_kernel_spmd(nc, in_maps, core_ids=list(range(NCORES)),
                               trace=trace, **kw)
    return res


def kernel(**inputs):
    in_maps = _prep_inputs(**inputs)
    res = run_on_hw(in_maps)
    return _combine(res.results)
